# revision 38
# baseline (speedup 1.0000x reference)
"""Trainium2 Bass kernel for GroupedQueryAttention (anti-causal mask variant).

Reference semantics (B=2, S=2048, D=4096, 32 Q heads, 4 KV heads, dk=128):
  Q = x@Wq, K = x@Wk, V = x@Wv (heads split), GQA repeat KV x8.
  scores = Q K^T / sqrt(dk); mask = triu(ones, k=1); scores = where(mask==0, -1e9, scores)
    -> keeps STRICT UPPER triangle (k > q, anti-causal). Rows with no valid key
       (q == S-1) become a uniform softmax over all S keys.
  out = softmax(scores) @ V; out = out @ Wo.

Sharding: 8 cores, 4 Q heads + their 1 shared KV head per core. Each core
computes a partial out = attn_heads @ Wo_rows_slice; host sums the 8 partials.

Per-core kernel design: matmuls run in bf16 (moving-operand dtype sets PE
speed: 1 cycle/row vs 4 for fp32) with fp32 PSUM accumulation; softmax
normalization stays fp32.
  - x^T chunks produced by PE transposes (quadrant-packed into one PSUM bank).
  - Q^T/K^T/V^T projections directly in [dk, seq] layout (lhsT = W chunk).
  - scores computed TRANSPOSED: sT[k, q] = K^T chunk (lhsT) x Q^T (rhs), so
    softmax denominator is a partition-dim sum (ones-matmul) and the AV matmul
    out^T[dk, q] = V chunk (lhsT) x P^T (rhs) accumulates with N=512 and lands
    already transposed for the Wo projection.
  - full (non-diagonal) score chunks are exp'd in PAIRS: both scores land in
    one 2-bank PSUM tile and a single 1024-wide ACTIVATE covers them,
    amortizing the Act engine's ~352-cycle per-instruction overhead.
  - masking: additive -1e9 on diagonal-band blocks (exp underflows to exact 0,
    matching the reference). Fully-masked blocks are skipped, and diagonal
    blocks are trimmed to the first (d+1)*128 query columns (the rest of the
    block is fully masked -> zero contribution). For the LAST q block the
    reference's fully-masked rows need uniform weights, so there the diag band
    uses a multiplicative mask to pin masked logits to exactly -30
    (exp(-30) ~ 9.4e-14), and the skipped blocks' contributions are added
    analytically: r += n_skip*128*exp(-30), out^T += exp(-30)*cumsum(V).
  - softmax normalization (DVE reciprocal, 3.4us) for q-block i is emitted
    after q-block i+1's score chunks so the DVE work overlaps the PE/Act
    stream instead of stalling it (po/pr PSUM double-buffering carries it).
"""

import sys
from contextlib import ExitStack

import numpy as np

for _p in ("/opt/trn_rl_repo",):
    if _p not in sys.path:
        sys.path.insert(0, _p)

import bass_rust
import concourse.bass as bass
import concourse.mybir as mybir
import concourse.tile as tile
from concourse.masks import make_identity


def _split_multiwaits(nc):
    """This walrus build encodes at most ONE sem wait per instruction.
    Tile's wait-assignment can attach several; hoist the extras onto fresh
    single-wait NoOps emitted immediately before the instruction on the same
    engine stream. Tile emits instructions in schedule order, so every wait's
    producer precedes the waiting instruction in-stream and the stall cannot
    deadlock."""
    for fn in nc.m.functions:
        for blk in fn.blocks:
            newlist = []
            for ins in blk.instructions:
                si = ins.sync_info
                n = len(si.on_wait) if si is not None else 0
                if n > 1:
                    waits = list(si.on_wait)
                    for j, w in enumerate(waits[:-1]):
                        nop = mybir.InstNoOp(
                            name=f"{ins.name}-hw{j}", engine=ins.engine,
                            ins=[], outs=[],
                            sync_info=bass_rust.SyncInfo(on_wait=[w],
                                                         on_update=[]))
                        nc.register_instruction(nop, overwrite=True)
                        newlist.append(nop)
                    si.on_wait = waits[-1:]
                newlist.append(ins)
            blk.instructions = newlist

B, S, D = 2, 2048, 4096
NQ, NKV, DK = 32, 4, 128
NCORES = 8
HPC = NQ // NCORES          # 4 q heads per core
DKC = HPC * DK              # 512 proj cols per core
SCALE = 1.0 / float(np.sqrt(DK))
NEGBIG = -1e9
MV = 30.0                   # masked logit magnitude (post-scale)
MASKED_PRE = -MV / SCALE    # pre-scale fill so exp(scale*fill) == exp(-30)
EXP_M = float(np.exp(-MV))
QB = 512                    # q block (matmul moving free dim)
KC = 128                    # k chunk (PE contraction/partition dim)
F32 = mybir.dt.float32
BF16 = mybir.dt.bfloat16
EXP = mybir.ActivationFunctionType.Exp
LN = mybir.ActivationFunctionType.Ln


def build_program(s=S):
    """Build the per-core Bass/Tile program. Same program for all 8 cores
    (SPMD); per-core weight slices are supplied via the input maps."""
    nqb = s // QB            # q blocks
    nkc = s // KC            # k chunks
    nd = D // KC             # D contraction chunks (32)
    ndq = 4                  # x loaded in 4 column quarters
    dq = D // ndq            # 1024

    nc = bass.Bass("TRN2", target_bir_lowering=False, debug=False,
                   num_devices=NCORES)
    x = nc.dram_tensor("x", [B, s, D], BF16, kind="ExternalInput").ap()
    wq = nc.dram_tensor("wq", [D, DKC], BF16, kind="ExternalInput").ap()
    wk = nc.dram_tensor("wk", [D, DK], BF16, kind="ExternalInput").ap()
    wv = nc.dram_tensor("wv", [D, DK], BF16, kind="ExternalInput").ap()
    wo = nc.dram_tensor("wo", [DKC, D], BF16, kind="ExternalInput").ap()
    mka = nc.dram_tensor("maskadd", [4, KC, QB], F32, kind="ExternalInput").ap()
    mkm = nc.dram_tensor("maskmul", [4, KC, QB], F32, kind="ExternalInput").ap()
    mkb = nc.dram_tensor("maskbias", [4, KC, QB], F32, kind="ExternalInput").ap()
    cid = nc.dram_tensor("cident", [KC, KC], BF16, kind="ExternalInput").ap()
    con = nc.dram_tensor("cones", [KC, KC], BF16, kind="ExternalInput").ap()
    crb = nc.dram_tensor("crbc", [KC, 1], F32, kind="ExternalInput").ap()
    out = nc.dram_tensor("out", [B, s, D], BF16, kind="ExternalOutput").ap()

    xf = x.rearrange("b s d -> (b s) d")
    of = out.rearrange("b s d -> (b s) d")
    nnb = D // QB            # 8 column blocks of Wo

    with tile.TileContext(nc) as tc, ExitStack() as ctx:
        # constants come from DRAM so no gpsimd/DVE init gates the first
        # transposes (make_identity on gpsimd cost ~8us of startup)
        consts = ctx.enter_context(tc.tile_pool(name="consts", bufs=1))
        ident = consts.tile([128, 128], BF16, name="ident", tag="ident")
        nc.sync.dma_start(out=ident, in_=cid)
        ones = consts.tile([128, 128], BF16, name="ones", tag="ones")
        nc.sync.dma_start(out=ones, in_=con)
        rbc = consts.tile([128, 1], F32, name="rbc", tag="rbc")
        nc.sync.dma_start(out=rbc, in_=crb)

        # ---- weights: tiles resident for both batches; the dma_starts are
        # emitted interleaved with batch-0's projection loop so the serial
        # DMA queue serves the x tiles (needed first) before the bulk of the
        # weights, instead of stalling the PE ~60us at kernel start.
        wpool = ctx.enter_context(tc.tile_pool(name="wqkv", bufs=1))
        wq_t = wpool.tile([128, nd, DKC], BF16, name="wq_t", tag="wq_t")
        wq_r = wq.rearrange("(c p) n -> p c n", p=128)
        wk_t = wpool.tile([128, nd, DK], BF16, name="wk_t", tag="wk_t")
        wk_r = wk.rearrange("(c p) n -> p c n", p=128)
        wv_t = wpool.tile([128, nd, DK], BF16, name="wv_t", tag="wv_t")
        wv_r = wv.rearrange("(c p) n -> p c n", p=128)
        wo_t = wpool.tile([128, HPC, nnb, QB], BF16, name="wo_t", tag="wo_t")
        wo_r = wo.rearrange("(c p) (nb n) -> p c nb n", p=128, n=QB)
        mpool = ctx.enter_context(tc.tile_pool(name="masks", bufs=1))
        ma_t = mpool.tile([128, 4, QB], F32, name="ma_t", tag="ma_t")
        mm_t = mpool.tile([128, 4, QB], F32, name="mm_t", tag="mm_t")
        mb_t = mpool.tile([128, 4, QB], F32, name="mb_t", tag="mb_t")

        # (the DMA XBAR transpose was tried for x^T and is 10-20x too slow:
        # it shreds into 256B descriptors; PE transposes stay)
        xpool = ctx.enter_context(tc.tile_pool(name="xload", bufs=10))
        xprefetch = {}

        def x_tile(b, qb, dqi, rt):
            key = (b, qb, dqi, rt)
            if key in xprefetch:
                return xprefetch.pop(key)
            xt_ = xpool.tile([128, dq], BF16, name="xt", tag="xt")
            row0 = b * s + qb * QB + rt * 128
            nc.sync.dma_start(
                out=xt_, in_=xf[row0:row0 + 128, dqi * dq:(dqi + 1) * dq])
            return xt_

        for b in range(B):
            with ExitStack() as bctx:
                bpool = bctx.enter_context(tc.tile_pool(name=f"bp{b}", bufs=1))
                qt = [bpool.tile([128, s], BF16, name=f"qt{b}_{h}", tag=f"qt{h}")
                      for h in range(HPC)]
                kt = bpool.tile([128, s], BF16, name=f"kt{b}", tag="kt")
                vt = bpool.tile([128, s], BF16, name=f"vt{b}", tag="vt")
                vn = bpool.tile([128, s], BF16, name=f"vn{b}", tag="vn")

                # ---------- projection phase: Q^T, K^T, V^T ----------
                with ExitStack() as pctx:
                    xtp = pctx.enter_context(tc.tile_pool(name="xtsb", bufs=6))
                    ppool = pctx.enter_context(
                        tc.tile_pool(name="projpsum", bufs=1, space="PSUM"))
                    tpool = pctx.enter_context(
                        tc.tile_pool(name="trpsum", bufs=2, space="PSUM"))

                    for qb in range(nqb):
                        pq = [ppool.tile([128, QB], F32, name=f"pq{h}", tag=f"pq{h}")
                              for h in range(HPC)]
                        pk = ppool.tile([128, QB], F32, name="pk", tag="pk")
                        pv = ppool.tile([128, QB], F32, name="pv", tag="pv")
                        for dqi in range(ndq):
                            xts = [x_tile(b, qb, dqi, rt) for rt in range(4)]
                            if b == 0 and qb == 0:
                                # weight chunks for this dqi, behind its x tiles
                                for kcg in range(dqi * 8, (dqi + 1) * 8):
                                    nc.sync.dma_start(out=wq_t[:, kcg, :],
                                                      in_=wq_r[:, kcg, :])
                                dsl = slice(dqi * 8, (dqi + 1) * 8)
                                nc.sync.dma_start(out=wk_t[:, dsl, :],
                                                  in_=wk_r[:, dsl, :])
                                nc.sync.dma_start(out=wv_t[:, dsl, :],
                                                  in_=wv_r[:, dsl, :])
                            for kci in range(dq // KC):
                                kcg = dqi * (dq // KC) + kci
                                ptp = tpool.tile([128, QB], BF16, name="ptp", tag="ptp")
                                for rt in range(4):
                                    nc.tensor.transpose(
                                        ptp[:, rt * 128:(rt + 1) * 128],
                                        xts[rt][:, kci * 128:(kci + 1) * 128],
                                        ident)
                                xT = xtp.tile([128, QB], BF16, name="xT", tag="xT")
                                # alternate engines by kcg parity: consecutive
                                # ptp banks drain concurrently, so transposes
                                # (gated on bank reuse, tpool bufs=2) wait for
                                # a copy that started a full kci earlier
                                if kcg % 2 == 0:
                                    nc.vector.tensor_copy(xT, ptp)
                                else:
                                    nc.scalar.copy(xT, ptp)
                                st = kcg == 0
                                sp = kcg == nd - 1
                                for h in range(HPC):
                                    nc.tensor.matmul(
                                        pq[h], wq_t[:, kcg, h * 128:(h + 1) * 128],
                                        xT, start=st, stop=sp)
                                nc.tensor.matmul(pk, wk_t[:, kcg, :], xT,
                                                 start=st, stop=sp)
                                nc.tensor.matmul(pv, wv_t[:, kcg, :], xT,
                                                 start=st, stop=sp)
                        sl = slice(qb * QB, (qb + 1) * QB)
                        for h in range(HPC):
                            nc.any.tensor_copy(qt[h][:, sl], pq[h])
                        nc.any.tensor_copy(kt[:, sl], pk)
                        nc.any.tensor_copy(vt[:, sl], pv)
                        if b == 0 and qb == 0:
                            nc.sync.dma_start(
                                out=ma_t, in_=mka.rearrange("d p n -> p d n"))
                            nc.sync.dma_start(
                                out=mm_t, in_=mkm.rearrange("d p n -> p d n"))
                            nc.sync.dma_start(
                                out=mb_t, in_=mkb.rearrange("d p n -> p d n"))
                        if b == 0 and qb in (1, 2):
                            for c in (0, 1) if qb == 1 else (2, 3):
                                nc.sync.dma_start(out=wo_t[:, c, :, :],
                                                  in_=wo_r[:, c, :, :])

                # ---------- V^T -> V natural ----------
                with ExitStack() as vctx:
                    vpsum = vctx.enter_context(
                        tc.tile_pool(name="vtpsum", bufs=2, space="PSUM"))
                    for kc in range(nkc):
                        pvt = vpsum.tile([128, 128], BF16, name="pvt", tag="pvt")
                        nc.tensor.transpose(
                            pvt, vt[:, kc * 128:(kc + 1) * 128], ident)
                        nc.any.tensor_copy(vn[:, kc * 128:(kc + 1) * 128], pvt)

                # ---------- cv = exp(-30) * cumsum_V over skipped chunks ----
                # (head-independent; used by the last q block's uniform rows)
                nskip = 4 * (nqb - 1)
                cvpool = bctx.enter_context(tc.tile_pool(name="cvsb", bufs=1))
                cv = None
                if nskip > 0:
                    with ExitStack() as cctx:
                        cps = cctx.enter_context(
                            tc.tile_pool(name="cvpsum", bufs=1, space="PSUM"))
                        pc = cps.tile([128, 1], F32, name="pc", tag="pc")
                        for i in range(nskip):
                            nc.tensor.matmul(
                                pc, vn[:, i * 128:(i + 1) * 128], ones[:, 0:1],
                                start=(i == 0), stop=(i == nskip - 1))
                        cv = cvpool.tile([128, 1], F32, name="cv", tag="cv")
                        nc.scalar.mul(cv, pc, EXP_M)

                # ---------- attention ----------
                apool = bctx.enter_context(tc.tile_pool(name=f"att{b}", bufs=1))
                att = [apool.tile([128, s], BF16, name=f"att{b}_{h}", tag=f"att{h}")
                       for h in range(HPC)]
                with ExitStack() as actx:
                    aps = actx.enter_context(
                        tc.tile_pool(name="atpsum", bufs=2, space="PSUM"))
                    spool = actx.enter_context(tc.tile_pool(name="attsb", bufs=4))
                    npool = actx.enter_context(tc.tile_pool(name="nrmsb", bufs=2))

                    def emit_normalize(po, pr, h, qb, last):
                        # 1/r on the Act engine as exp(-ln(r)): the DVE
                        # reciprocal (3.4us, uninterruptible) kept blocking
                        # the mask-add FIFO and stalling the exp->AV chain.
                        # natural_log_exp_and_others serves Ln and Exp with
                        # no table switch.
                        qsl = slice(qb * QB, (qb + 1) * QB)
                        rr = npool.tile([128, QB], F32, name="rr", tag="rr")
                        lr = npool.tile([128, QB], F32, name="lr", tag="lr")
                        if last and nskip > 0:
                            nc.scalar.activation(lr, pr, LN, bias=rbc)
                            nc.scalar.activation(rr, lr, EXP, scale=-1.0)
                            tno = npool.tile([128, QB], F32, name="tno",
                                             tag="tno")
                            nc.scalar.add(tno, po, cv)
                            nc.vector.tensor_mul(att[h][:, qsl], tno, rr)
                        else:
                            nc.scalar.activation(lr, pr, LN)
                            nc.scalar.activation(rr, lr, EXP, scale=-1.0)
                            nc.vector.tensor_mul(att[h][:, qsl], po, rr)

                    # Two heads processed interleaved per q-block: each holds
                    # its own po/pr accumulators (4 PSUM banks) and shares the
                    # ps2 score-pair pool (4 banks) -> 8 exactly. The second
                    # head's matmul stream hides the first head's exp/mask
                    # latency, and the inline normalizes hide under the next
                    # q-block's score matmuls.
                    for h0 in (0, 2):
                        for qb in range(nqb):
                            last = qb == nqb - 1
                            q0 = qb * QB
                            # diag chunks PAIRED with widened n (d0->256,
                            # d2->512): the extra columns are fully masked so
                            # the additive -1e9 mask turns them into exact
                            # zeros, and each pair shares one DVE add + one
                            # Act exp instead of two of each.
                            steps = []   # each: list of (kc, n)
                            if not last:
                                steps.append([(4 * qb + 0, 256),
                                              (4 * qb + 1, 256)])
                                steps.append([(4 * qb + 2, QB),
                                              (4 * qb + 3, QB)])
                                rest = list(range(4 * qb + 4, nkc))
                            else:
                                rest = list(range(4 * qb, nkc))
                            for j in range(0, len(rest), 2):
                                steps.append([(kc, QB) for kc in rest[j:j + 2]])
                            nch = sum(len(st) for st in steps)
                            pos, prs, cis = {}, {}, {}
                            for hh in (h0, h0 + 1):
                                i = hh - h0
                                pos[hh] = aps.tile([128, QB], F32,
                                                   name=f"po{i}", tag=f"po{i}",
                                                   bufs=1)
                                prs[hh] = aps.tile([128, QB], F32,
                                                   name=f"pr{i}", tag=f"pr{i}",
                                                   bufs=1)
                                cis[hh] = 0
                            for st in steps:
                                for hh in (h0, h0 + 1):
                                    po, pr = pos[hh], prs[hh]
                                    ps2 = aps.tile([128, 2, QB], F32,
                                                   name="ps2", tag="ps2",
                                                   bufs=2)
                                    pt2 = spool.tile([128, 2, QB], BF16,
                                                     name="pt2", tag="pt2")
                                    for sj, (kc, n) in enumerate(st):
                                        nc.tensor.matmul(
                                            ps2[:, sj, 0:n],
                                            kt[:, kc * 128:(kc + 1) * 128],
                                            qt[hh][:, q0:q0 + n],
                                            start=True, stop=True)
                                    d0 = st[0][0] - 4 * qb
                                    if last:
                                        tm2 = spool.tile([128, 2, QB], F32,
                                                         name="tm2", tag="tm2")
                                        nw = len(st)
                                        nc.vector.tensor_mul(
                                            tm2[:, 0:nw, :], ps2[:, 0:nw, :],
                                            mm_t[:, d0:d0 + nw, :])
                                        nc.vector.tensor_add(
                                            tm2[:, 0:nw, :], tm2[:, 0:nw, :],
                                            mb_t[:, d0:d0 + nw, :])
                                        nc.scalar.activation(
                                            pt2[:, 0:nw, :], tm2[:, 0:nw, :],
                                            EXP, scale=SCALE)
                                    elif d0 < 4:
                                        # diag pair, both halves same width
                                        n = st[0][1]
                                        nw = len(st)
                                        tm2 = spool.tile([128, 2, QB], F32,
                                                         name="tm2", tag="tm2")
                                        nc.vector.tensor_add(
                                            tm2[:, 0:nw, 0:n],
                                            ps2[:, 0:nw, 0:n],
                                            ma_t[:, d0:d0 + nw, 0:n])
                                        nc.scalar.activation(
                                            pt2[:, 0:nw, 0:n],
                                            tm2[:, 0:nw, 0:n],
                                            EXP, scale=SCALE)
                                    else:
                                        nw = len(st)
                                        nc.scalar.activation(
                                            pt2[:, 0:nw, :], ps2[:, 0:nw, :],
                                            EXP, scale=SCALE)
                                    for sj, (kc, n) in enumerate(st):
                                        ci = cis[hh]
                                        nc.tensor.matmul(
                                            po[:, 0:n],
                                            vn[:, kc * 128:(kc + 1) * 128],
                                            pt2[:, sj, 0:n],
                                            start=(ci == 0),
                                            stop=(ci == nch - 1))
                                        nc.tensor.matmul(
                                            pr[:, 0:n], ones, pt2[:, sj, 0:n],
                                            start=(ci == 0),
                                            stop=(ci == nch - 1))
                                        cis[hh] += 1
                            for hh in (h0, h0 + 1):
                                emit_normalize(pos[hh], prs[hh], hh, qb, last)

                # ---------- output projection (partial: this core's heads) ----
                if b + 1 < B:
                    # prefetch next batch's first x tiles ahead of the 16MB of
                    # output-staging DMA writes this phase puts in the queue
                    for dqi in range(2):
                        for rt in range(4):
                            xprefetch[(b + 1, 0, dqi, rt)] = x_tile(
                                b + 1, 0, dqi, rt)
                with ExitStack() as wctx:
                    opsum = wctx.enter_context(
                        tc.tile_pool(name="opsum", bufs=4, space="PSUM"))
                    stpool = wctx.enter_context(tc.tile_pool(name="ostage", bufs=2))
                    for qti in range(s // 128):
                        stg = stpool.tile([128, D], BF16, name="stg", tag="stg")
                        for nb in range(nnb):
                            po2 = opsum.tile([128, QB], F32, name="po2", tag="po2")
                            for c in range(HPC):
                                nc.tensor.matmul(
                                    po2, att[c][:, qti * 128:(qti + 1) * 128],
                                    wo_t[:, c, nb, :],
                                    start=(c == 0), stop=(c == HPC - 1))
                            # split the PSUM->SBUF copy across Act and DVE:
                            # a single-engine 686ns copy paces below the
                            # 853ns matmul cadence and stalled the PE
                            h0 = nb * QB
                            nc.scalar.copy(stg[:, h0:h0 + QB // 2],
                                           po2[:, 0:QB // 2])
                            nc.vector.tensor_copy(stg[:, h0 + QB // 2:h0 + QB],
                                                  po2[:, QB // 2:QB])
                        row0 = b * s + qti * 128
                        nc.sync.dma_start(out=of[row0:row0 + 128, :], in_=stg)
    _split_multiwaits(nc)
    return nc


def make_masks():
    r = np.arange(KC)[:, None]
    c = np.arange(QB)[None, :]
    valid = [(r + 128 * d) > c for d in range(4)]   # k > q within block
    ma = np.stack([np.where(v, 0.0, NEGBIG) for v in valid]).astype(np.float32)
    mm = np.stack([v.astype(np.float32) for v in valid])
    mb = np.stack([np.where(v, 0.0, MASKED_PRE) for v in valid]).astype(np.float32)
    return ma, mm, mb


_PROG = {}


def _get_program(s=S):
    if s not in _PROG:
        _PROG[s] = build_program(s)
    return _PROG[s]


def _bf16(a):
    import ml_dtypes

    return np.ascontiguousarray(np.asarray(a, np.float32).astype(ml_dtypes.bfloat16))


_XCACHE = {}


def core_in_map(c, x, Wq, Wk, Wv, Wo):
    ma, mm, mb = make_masks()
    h0 = c * HPC
    kv = (c * HPC) // (NQ // NKV)
    key = id(x)
    if key not in _XCACHE:
        _XCACHE.clear()
        _XCACHE[key] = _bf16(x)
    nqb = S // QB
    return {
        "x": _XCACHE[key],
        "wq": _bf16(np.asarray(Wq, np.float32)[:, h0 * DK:(h0 + HPC) * DK]),
        "wk": _bf16(np.asarray(Wk, np.float32)[:, kv * DK:(kv + 1) * DK]),
        "wv": _bf16(np.asarray(Wv, np.float32)[:, kv * DK:(kv + 1) * DK]),
        "wo": _bf16(np.asarray(Wo, np.float32)[h0 * DK:(h0 + HPC) * DK, :]),
        "maskadd": ma,
        "maskmul": mm,
        "maskbias": mb,
        "cident": _bf16(np.eye(KC, dtype=np.float32)),
        "cones": _bf16(np.ones((KC, KC), np.float32)),
        "crbc": np.full((KC, 1), 4 * (nqb - 1) * 128 * EXP_M, np.float32),
    }


def combine_outputs(results, shape):
    acc = np.zeros(shape, np.float64)
    for r in results:
        acc += np.asarray(r["out"], np.float64)
    return acc.astype(np.float32)


def kernel(x, Wq, Wk, Wv, Wo, **kw):
    from concourse.bass_utils import run_bass_kernel_spmd

    nc = _get_program(np.asarray(x).shape[1])
    in_maps = [core_in_map(c, x, Wq, Wk, Wv, Wo) for c in range(NCORES)]
    res = run_bass_kernel_spmd(nc, in_maps, core_ids=list(range(NCORES)), **kw)
    return combine_outputs(res.results, np.asarray(x).shape)


# revision 40
# speedup vs baseline: 1.0325x; 1.0325x over previous
"""Trainium2 Bass kernel for GroupedQueryAttention (anti-causal mask variant).

Reference semantics (B=2, S=2048, D=4096, 32 Q heads, 4 KV heads, dk=128):
  Q = x@Wq, K = x@Wk, V = x@Wv (heads split), GQA repeat KV x8.
  scores = Q K^T / sqrt(dk); mask = triu(ones, k=1); scores = where(mask==0, -1e9, scores)
    -> keeps STRICT UPPER triangle (k > q, anti-causal). Rows with no valid key
       (q == S-1) become a uniform softmax over all S keys.
  out = softmax(scores) @ V; out = out @ Wo.

Sharding: 8 cores, 4 Q heads + their 1 shared KV head per core. Each core
computes a partial out = attn_heads @ Wo_rows_slice; host sums the 8 partials.

Per-core kernel design: matmuls run in bf16 (moving-operand dtype sets PE
speed: 1 cycle/row vs 4 for fp32) with fp32 PSUM accumulation; softmax
normalization stays fp32.
  - x^T chunks produced by PE transposes (quadrant-packed into one PSUM bank).
  - Q^T/K^T/V^T projections directly in [dk, seq] layout (lhsT = W chunk).
  - scores computed TRANSPOSED: sT[k, q] = K^T chunk (lhsT) x Q^T (rhs), so
    softmax denominator is a partition-dim sum (ones-matmul) and the AV matmul
    out^T[dk, q] = V chunk (lhsT) x P^T (rhs) accumulates with N=512 and lands
    already transposed for the Wo projection.
  - full (non-diagonal) score chunks are exp'd in PAIRS: both scores land in
    one 2-bank PSUM tile and a single 1024-wide ACTIVATE covers them,
    amortizing the Act engine's ~352-cycle per-instruction overhead.
  - masking: additive -1e9 on diagonal-band blocks (exp underflows to exact 0,
    matching the reference). Fully-masked blocks are skipped, and diagonal
    blocks are trimmed to the first (d+1)*128 query columns (the rest of the
    block is fully masked -> zero contribution). For the LAST q block the
    reference's fully-masked rows need uniform weights, so there the diag band
    uses a multiplicative mask to pin masked logits to exactly -30
    (exp(-30) ~ 9.4e-14), and the skipped blocks' contributions are added
    analytically: r += n_skip*128*exp(-30), out^T += exp(-30)*cumsum(V).
  - softmax normalization (DVE reciprocal, 3.4us) for q-block i is emitted
    after q-block i+1's score chunks so the DVE work overlaps the PE/Act
    stream instead of stalling it (po/pr PSUM double-buffering carries it).
"""

import sys
from contextlib import ExitStack

import numpy as np

for _p in ("/opt/trn_rl_repo",):
    if _p not in sys.path:
        sys.path.insert(0, _p)

import bass_rust
import concourse.bass as bass
import concourse.mybir as mybir
import concourse.tile as tile
from concourse.masks import make_identity


def _split_multiwaits(nc):
    """This walrus build encodes at most ONE sem wait per instruction.
    Tile's wait-assignment can attach several; hoist the extras onto fresh
    single-wait NoOps emitted immediately before the instruction on the same
    engine stream. Tile emits instructions in schedule order, so every wait's
    producer precedes the waiting instruction in-stream and the stall cannot
    deadlock."""
    for fn in nc.m.functions:
        for blk in fn.blocks:
            newlist = []
            for ins in blk.instructions:
                si = ins.sync_info
                n = len(si.on_wait) if si is not None else 0
                if n > 1:
                    waits = list(si.on_wait)
                    for j, w in enumerate(waits[:-1]):
                        nop = mybir.InstNoOp(
                            name=f"{ins.name}-hw{j}", engine=ins.engine,
                            ins=[], outs=[],
                            sync_info=bass_rust.SyncInfo(on_wait=[w],
                                                         on_update=[]))
                        nc.register_instruction(nop, overwrite=True)
                        newlist.append(nop)
                    si.on_wait = waits[-1:]
                newlist.append(ins)
            blk.instructions = newlist

B, S, D = 2, 2048, 4096
NQ, NKV, DK = 32, 4, 128
NCORES = 8
HPC = NQ // NCORES          # 4 q heads per core
DKC = HPC * DK              # 512 proj cols per core
SCALE = 1.0 / float(np.sqrt(DK))
NEGBIG = -1e9
MV = 30.0                   # masked logit magnitude (post-scale)
MASKED_PRE = -MV / SCALE    # pre-scale fill so exp(scale*fill) == exp(-30)
EXP_M = float(np.exp(-MV))
QB = 512                    # q block (matmul moving free dim)
KC = 128                    # k chunk (PE contraction/partition dim)
F32 = mybir.dt.float32
BF16 = mybir.dt.bfloat16
EXP = mybir.ActivationFunctionType.Exp
LN = mybir.ActivationFunctionType.Ln


def build_program(s=S):
    """Build the per-core Bass/Tile program. Same program for all 8 cores
    (SPMD); per-core weight slices are supplied via the input maps."""
    nqb = s // QB            # q blocks
    nkc = s // KC            # k chunks
    nd = D // KC             # D contraction chunks (32)
    ndq = 4                  # x loaded in 4 column quarters
    dq = D // ndq            # 1024

    nc = bass.Bass("TRN2", target_bir_lowering=False, debug=False,
                   num_devices=NCORES)
    x = nc.dram_tensor("x", [B, s, D], BF16, kind="ExternalInput").ap()
    wq = nc.dram_tensor("wq", [D, DKC], BF16, kind="ExternalInput").ap()
    wk = nc.dram_tensor("wk", [D, DK], BF16, kind="ExternalInput").ap()
    wv = nc.dram_tensor("wv", [D, DK], BF16, kind="ExternalInput").ap()
    wo = nc.dram_tensor("wo", [DKC, D], BF16, kind="ExternalInput").ap()
    mka = nc.dram_tensor("maskadd", [4, KC, QB], F32, kind="ExternalInput").ap()
    mkm = nc.dram_tensor("maskmul", [4, KC, QB], F32, kind="ExternalInput").ap()
    mkb = nc.dram_tensor("maskbias", [4, KC, QB], F32, kind="ExternalInput").ap()
    cid = nc.dram_tensor("cident", [KC, KC], BF16, kind="ExternalInput").ap()
    con = nc.dram_tensor("cones", [KC, KC], BF16, kind="ExternalInput").ap()
    crb = nc.dram_tensor("crbc", [KC, 1], F32, kind="ExternalInput").ap()
    out = nc.dram_tensor("out", [B, s, D], BF16, kind="ExternalOutput").ap()

    xf = x.rearrange("b s d -> (b s) d")
    of = out.rearrange("b s d -> (b s) d")
    nnb = D // QB            # 8 column blocks of Wo

    with tile.TileContext(nc) as tc, ExitStack() as ctx:
        # constants come from DRAM so no gpsimd/DVE init gates the first
        # transposes (make_identity on gpsimd cost ~8us of startup)
        consts = ctx.enter_context(tc.tile_pool(name="consts", bufs=1))
        ident = consts.tile([128, 128], BF16, name="ident", tag="ident")
        nc.sync.dma_start(out=ident, in_=cid)
        ones = consts.tile([128, 128], BF16, name="ones", tag="ones")
        nc.sync.dma_start(out=ones, in_=con)
        rbc = consts.tile([128, 1], F32, name="rbc", tag="rbc")
        nc.sync.dma_start(out=rbc, in_=crb)

        # ---- weights: tiles resident for both batches; the dma_starts are
        # emitted interleaved with batch-0's projection loop so the serial
        # DMA queue serves the x tiles (needed first) before the bulk of the
        # weights, instead of stalling the PE ~60us at kernel start.
        wpool = ctx.enter_context(tc.tile_pool(name="wqkv", bufs=1))
        wq_t = wpool.tile([128, nd, DKC], BF16, name="wq_t", tag="wq_t")
        wq_r = wq.rearrange("(c p) n -> p c n", p=128)
        wk_t = wpool.tile([128, nd, DK], BF16, name="wk_t", tag="wk_t")
        wk_r = wk.rearrange("(c p) n -> p c n", p=128)
        wv_t = wpool.tile([128, nd, DK], BF16, name="wv_t", tag="wv_t")
        wv_r = wv.rearrange("(c p) n -> p c n", p=128)
        wo_t = wpool.tile([128, HPC, nnb, QB], BF16, name="wo_t", tag="wo_t")
        wo_r = wo.rearrange("(c p) (nb n) -> p c nb n", p=128, n=QB)
        mpool = ctx.enter_context(tc.tile_pool(name="masks", bufs=1))
        ma_t = mpool.tile([128, 4, QB], F32, name="ma_t", tag="ma_t")
        mm_t = mpool.tile([128, 4, QB], F32, name="mm_t", tag="mm_t")
        mb_t = mpool.tile([128, 4, QB], F32, name="mb_t", tag="mb_t")

        # (the DMA XBAR transpose was tried for x^T and is 10-20x too slow:
        # it shreds into 256B descriptors; PE transposes stay)
        xpool = ctx.enter_context(tc.tile_pool(name="xload", bufs=10))
        xprefetch = {}

        def x_tile(b, qb, dqi, rt):
            key = (b, qb, dqi, rt)
            if key in xprefetch:
                return xprefetch.pop(key)
            xt_ = xpool.tile([128, dq], BF16, name="xt", tag="xt")
            row0 = b * s + qb * QB + rt * 128
            nc.sync.dma_start(
                out=xt_, in_=xf[row0:row0 + 128, dqi * dq:(dqi + 1) * dq])
            return xt_

        for b in range(B):
            with ExitStack() as bctx:
                bpool = bctx.enter_context(tc.tile_pool(name=f"bp{b}", bufs=1))
                qt = [bpool.tile([128, s], BF16, name=f"qt{b}_{h}", tag=f"qt{h}")
                      for h in range(HPC)]
                kt = bpool.tile([128, s], BF16, name=f"kt{b}", tag="kt")
                vt = bpool.tile([128, s], BF16, name=f"vt{b}", tag="vt")
                vn = bpool.tile([128, s], BF16, name=f"vn{b}", tag="vn")

                # ---------- projection phase: Q^T, K^T, V^T ----------
                with ExitStack() as pctx:
                    xtp = pctx.enter_context(tc.tile_pool(name="xtsb", bufs=6))
                    ppool = pctx.enter_context(
                        tc.tile_pool(name="projpsum", bufs=1, space="PSUM"))
                    tpool = pctx.enter_context(
                        tc.tile_pool(name="trpsum", bufs=2, space="PSUM"))

                    for qb in range(nqb):
                        pq = [ppool.tile([128, QB], F32, name=f"pq{h}", tag=f"pq{h}")
                              for h in range(HPC)]
                        pk = ppool.tile([128, QB], F32, name="pk", tag="pk")
                        pv = ppool.tile([128, QB], F32, name="pv", tag="pv")
                        for dqi in range(ndq):
                            xts = [x_tile(b, qb, dqi, rt) for rt in range(4)]
                            if b == 0 and qb == 0:
                                # weight chunks for this dqi, behind its x tiles
                                for kcg in range(dqi * 8, (dqi + 1) * 8):
                                    nc.sync.dma_start(out=wq_t[:, kcg, :],
                                                      in_=wq_r[:, kcg, :])
                                dsl = slice(dqi * 8, (dqi + 1) * 8)
                                nc.sync.dma_start(out=wk_t[:, dsl, :],
                                                  in_=wk_r[:, dsl, :])
                                nc.sync.dma_start(out=wv_t[:, dsl, :],
                                                  in_=wv_r[:, dsl, :])
                            for kci in range(dq // KC):
                                kcg = dqi * (dq // KC) + kci
                                ptp = tpool.tile([128, QB], BF16, name="ptp", tag="ptp")
                                for rt in range(4):
                                    nc.tensor.transpose(
                                        ptp[:, rt * 128:(rt + 1) * 128],
                                        xts[rt][:, kci * 128:(kci + 1) * 128],
                                        ident)
                                xT = xtp.tile([128, QB], BF16, name="xT", tag="xT")
                                nc.any.tensor_copy(xT, ptp)
                                st = kcg == 0
                                sp = kcg == nd - 1
                                for h in range(HPC):
                                    nc.tensor.matmul(
                                        pq[h], wq_t[:, kcg, h * 128:(h + 1) * 128],
                                        xT, start=st, stop=sp)
                                nc.tensor.matmul(pk, wk_t[:, kcg, :], xT,
                                                 start=st, stop=sp)
                                nc.tensor.matmul(pv, wv_t[:, kcg, :], xT,
                                                 start=st, stop=sp)
                        sl = slice(qb * QB, (qb + 1) * QB)
                        for h in range(HPC):
                            nc.any.tensor_copy(qt[h][:, sl], pq[h])
                        nc.any.tensor_copy(kt[:, sl], pk)
                        nc.any.tensor_copy(vt[:, sl], pv)
                        if b == 0 and qb == 0:
                            nc.sync.dma_start(
                                out=ma_t, in_=mka.rearrange("d p n -> p d n"))
                            nc.sync.dma_start(
                                out=mm_t, in_=mkm.rearrange("d p n -> p d n"))
                            nc.sync.dma_start(
                                out=mb_t, in_=mkb.rearrange("d p n -> p d n"))
                        if b == 0 and qb in (1, 2):
                            for c in (0, 1) if qb == 1 else (2, 3):
                                nc.sync.dma_start(out=wo_t[:, c, :, :],
                                                  in_=wo_r[:, c, :, :])

                # ---------- V^T -> V natural ----------
                with ExitStack() as vctx:
                    vpsum = vctx.enter_context(
                        tc.tile_pool(name="vtpsum", bufs=2, space="PSUM"))
                    for kc in range(nkc):
                        pvt = vpsum.tile([128, 128], BF16, name="pvt", tag="pvt")
                        nc.tensor.transpose(
                            pvt, vt[:, kc * 128:(kc + 1) * 128], ident)
                        nc.any.tensor_copy(vn[:, kc * 128:(kc + 1) * 128], pvt)

                # ---------- cv = exp(-30) * cumsum_V over skipped chunks ----
                # (head-independent; used by the last q block's uniform rows)
                nskip = 4 * (nqb - 1)
                cvpool = bctx.enter_context(tc.tile_pool(name="cvsb", bufs=1))
                cv = None
                if nskip > 0:
                    with ExitStack() as cctx:
                        cps = cctx.enter_context(
                            tc.tile_pool(name="cvpsum", bufs=1, space="PSUM"))
                        pc = cps.tile([128, 1], F32, name="pc", tag="pc")
                        for i in range(nskip):
                            nc.tensor.matmul(
                                pc, vn[:, i * 128:(i + 1) * 128], ones[:, 0:1],
                                start=(i == 0), stop=(i == nskip - 1))
                        cv = cvpool.tile([128, 1], F32, name="cv", tag="cv")
                        nc.scalar.mul(cv, pc, EXP_M)

                # ---------- attention ----------
                apool = bctx.enter_context(tc.tile_pool(name=f"att{b}", bufs=1))
                att = [apool.tile([128, s], BF16, name=f"att{b}_{h}", tag=f"att{h}")
                       for h in range(HPC)]
                with ExitStack() as actx:
                    aps = actx.enter_context(
                        tc.tile_pool(name="atpsum", bufs=2, space="PSUM"))
                    spool = actx.enter_context(tc.tile_pool(name="attsb", bufs=4))
                    npool = actx.enter_context(tc.tile_pool(name="nrmsb", bufs=2))

                    def emit_normalize(po, pr, h, qb, last):
                        # 1/r on the Act engine as exp(-ln(r)): the DVE
                        # reciprocal (3.4us, uninterruptible) kept blocking
                        # the mask-add FIFO and stalling the exp->AV chain.
                        # natural_log_exp_and_others serves Ln and Exp with
                        # no table switch.
                        qsl = slice(qb * QB, (qb + 1) * QB)
                        rr = npool.tile([128, QB], F32, name="rr", tag="rr")
                        lr = npool.tile([128, QB], F32, name="lr", tag="lr")
                        if last and nskip > 0:
                            nc.scalar.activation(lr, pr, LN, bias=rbc)
                            nc.scalar.activation(rr, lr, EXP, scale=-1.0)
                            tno = npool.tile([128, QB], F32, name="tno",
                                             tag="tno")
                            nc.scalar.add(tno, po, cv)
                            nc.vector.tensor_mul(att[h][:, qsl], tno, rr)
                        else:
                            nc.scalar.activation(lr, pr, LN)
                            nc.scalar.activation(rr, lr, EXP, scale=-1.0)
                            nc.vector.tensor_mul(att[h][:, qsl], po, rr)

                    # Two heads processed interleaved per q-block: each holds
                    # its own po/pr accumulators (4 PSUM banks) and shares the
                    # ps2 score-pair pool (4 banks) -> 8 exactly. The second
                    # head's matmul stream hides the first head's exp/mask
                    # latency, and the inline normalizes hide under the next
                    # q-block's score matmuls.
                    for h0 in (0, 2):
                        for qb in range(nqb):
                            last = qb == nqb - 1
                            q0 = qb * QB
                            # diag chunks PAIRED with widened n (d0->256,
                            # d2->512): the extra columns are fully masked so
                            # the additive -1e9 mask turns them into exact
                            # zeros, and each pair shares one DVE add + one
                            # Act exp instead of two of each.
                            steps = []   # each: list of (kc, n)
                            if not last:
                                steps.append([(4 * qb + 0, 256),
                                              (4 * qb + 1, 256)])
                                steps.append([(4 * qb + 2, QB),
                                              (4 * qb + 3, QB)])
                                rest = list(range(4 * qb + 4, nkc))
                            else:
                                rest = list(range(4 * qb, nkc))
                            for j in range(0, len(rest), 2):
                                steps.append([(kc, QB) for kc in rest[j:j + 2]])
                            nch = sum(len(st) for st in steps)
                            pos, prs, cis = {}, {}, {}
                            for hh in (h0, h0 + 1):
                                i = hh - h0
                                pos[hh] = aps.tile([128, QB], F32,
                                                   name=f"po{i}", tag=f"po{i}",
                                                   bufs=1)
                                prs[hh] = aps.tile([128, QB], F32,
                                                   name=f"pr{i}", tag=f"pr{i}",
                                                   bufs=1)
                                cis[hh] = 0
                            def emit_avs(pend):
                                for hh, st, pt2 in pend:
                                    po, pr = pos[hh], prs[hh]
                                    for sj, (kc, n) in enumerate(st):
                                        ci = cis[hh]
                                        nc.tensor.matmul(
                                            po[:, 0:n],
                                            vn[:, kc * 128:(kc + 1) * 128],
                                            pt2[:, sj, 0:n],
                                            start=(ci == 0),
                                            stop=(ci == nch - 1))
                                        nc.tensor.matmul(
                                            pr[:, 0:n], ones, pt2[:, sj, 0:n],
                                            start=(ci == 0),
                                            stop=(ci == nch - 1))
                                        cis[hh] += 1

                            # AV/r matmuls are deferred one step: the in-order
                            # PE stream then has a full step of score matmuls
                            # between a pair's scores and its exp-dependent
                            # AVs, hiding the Act/DVE latency instead of
                            # stalling on it. pt2 bufs=4 holds 2 steps.
                            pend = []
                            for st in steps:
                                cur = []
                                for hh in (h0, h0 + 1):
                                    ps2 = aps.tile([128, 2, QB], F32,
                                                   name="ps2", tag="ps2",
                                                   bufs=2)
                                    pt2 = spool.tile([128, 2, QB], BF16,
                                                     name="pt2", tag="pt2")
                                    for sj, (kc, n) in enumerate(st):
                                        nc.tensor.matmul(
                                            ps2[:, sj, 0:n],
                                            kt[:, kc * 128:(kc + 1) * 128],
                                            qt[hh][:, q0:q0 + n],
                                            start=True, stop=True)
                                    d0 = st[0][0] - 4 * qb
                                    if last:
                                        tm2 = spool.tile([128, 2, QB], F32,
                                                         name="tm2", tag="tm2")
                                        nw = len(st)
                                        nc.vector.tensor_mul(
                                            tm2[:, 0:nw, :], ps2[:, 0:nw, :],
                                            mm_t[:, d0:d0 + nw, :])
                                        nc.vector.tensor_add(
                                            tm2[:, 0:nw, :], tm2[:, 0:nw, :],
                                            mb_t[:, d0:d0 + nw, :])
                                        nc.scalar.activation(
                                            pt2[:, 0:nw, :], tm2[:, 0:nw, :],
                                            EXP, scale=SCALE)
                                    elif d0 < 4:
                                        # diag pair, both halves same width
                                        n = st[0][1]
                                        nw = len(st)
                                        tm2 = spool.tile([128, 2, QB], F32,
                                                         name="tm2", tag="tm2")
                                        nc.vector.tensor_add(
                                            tm2[:, 0:nw, 0:n],
                                            ps2[:, 0:nw, 0:n],
                                            ma_t[:, d0:d0 + nw, 0:n])
                                        nc.scalar.activation(
                                            pt2[:, 0:nw, 0:n],
                                            tm2[:, 0:nw, 0:n],
                                            EXP, scale=SCALE)
                                    else:
                                        nw = len(st)
                                        nc.scalar.activation(
                                            pt2[:, 0:nw, :], ps2[:, 0:nw, :],
                                            EXP, scale=SCALE)
                                    cur.append((hh, st, pt2))
                                emit_avs(pend)
                                pend = cur
                            emit_avs(pend)
                            for hh in (h0, h0 + 1):
                                emit_normalize(pos[hh], prs[hh], hh, qb, last)

                # ---------- output projection (partial: this core's heads) ----
                if b + 1 < B:
                    # prefetch next batch's first x tiles ahead of the 16MB of
                    # output-staging DMA writes this phase puts in the queue
                    for dqi in range(2):
                        for rt in range(4):
                            xprefetch[(b + 1, 0, dqi, rt)] = x_tile(
                                b + 1, 0, dqi, rt)
                with ExitStack() as wctx:
                    opsum = wctx.enter_context(
                        tc.tile_pool(name="opsum", bufs=4, space="PSUM"))
                    stpool = wctx.enter_context(tc.tile_pool(name="ostage", bufs=2))
                    for qti in range(s // 128):
                        stg = stpool.tile([128, D], BF16, name="stg", tag="stg")
                        for nb in range(nnb):
                            po2 = opsum.tile([128, QB], F32, name="po2", tag="po2")
                            for c in range(HPC):
                                nc.tensor.matmul(
                                    po2, att[c][:, qti * 128:(qti + 1) * 128],
                                    wo_t[:, c, nb, :],
                                    start=(c == 0), stop=(c == HPC - 1))
                            # split the PSUM->SBUF copy across Act and DVE:
                            # a single-engine 686ns copy paces below the
                            # 853ns matmul cadence and stalled the PE
                            h0 = nb * QB
                            nc.scalar.copy(stg[:, h0:h0 + QB // 2],
                                           po2[:, 0:QB // 2])
                            nc.vector.tensor_copy(stg[:, h0 + QB // 2:h0 + QB],
                                                  po2[:, QB // 2:QB])
                        row0 = b * s + qti * 128
                        nc.sync.dma_start(out=of[row0:row0 + 128, :], in_=stg)
    _split_multiwaits(nc)
    return nc


def make_masks():
    r = np.arange(KC)[:, None]
    c = np.arange(QB)[None, :]
    valid = [(r + 128 * d) > c for d in range(4)]   # k > q within block
    ma = np.stack([np.where(v, 0.0, NEGBIG) for v in valid]).astype(np.float32)
    mm = np.stack([v.astype(np.float32) for v in valid])
    mb = np.stack([np.where(v, 0.0, MASKED_PRE) for v in valid]).astype(np.float32)
    return ma, mm, mb


_PROG = {}


def _get_program(s=S):
    if s not in _PROG:
        _PROG[s] = build_program(s)
    return _PROG[s]


def _bf16(a):
    import ml_dtypes

    return np.ascontiguousarray(np.asarray(a, np.float32).astype(ml_dtypes.bfloat16))


_XCACHE = {}


def core_in_map(c, x, Wq, Wk, Wv, Wo):
    ma, mm, mb = make_masks()
    h0 = c * HPC
    kv = (c * HPC) // (NQ // NKV)
    key = id(x)
    if key not in _XCACHE:
        _XCACHE.clear()
        _XCACHE[key] = _bf16(x)
    nqb = S // QB
    return {
        "x": _XCACHE[key],
        "wq": _bf16(np.asarray(Wq, np.float32)[:, h0 * DK:(h0 + HPC) * DK]),
        "wk": _bf16(np.asarray(Wk, np.float32)[:, kv * DK:(kv + 1) * DK]),
        "wv": _bf16(np.asarray(Wv, np.float32)[:, kv * DK:(kv + 1) * DK]),
        "wo": _bf16(np.asarray(Wo, np.float32)[h0 * DK:(h0 + HPC) * DK, :]),
        "maskadd": ma,
        "maskmul": mm,
        "maskbias": mb,
        "cident": _bf16(np.eye(KC, dtype=np.float32)),
        "cones": _bf16(np.ones((KC, KC), np.float32)),
        "crbc": np.full((KC, 1), 4 * (nqb - 1) * 128 * EXP_M, np.float32),
    }


def combine_outputs(results, shape):
    acc = np.zeros(shape, np.float64)
    for r in results:
        acc += np.asarray(r["out"], np.float64)
    return acc.astype(np.float32)


def kernel(x, Wq, Wk, Wv, Wo, **kw):
    from concourse.bass_utils import run_bass_kernel_spmd

    nc = _get_program(np.asarray(x).shape[1])
    in_maps = [core_in_map(c, x, Wq, Wk, Wv, Wo) for c in range(NCORES)]
    res = run_bass_kernel_spmd(nc, in_maps, core_ids=list(range(NCORES)), **kw)
    return combine_outputs(res.results, np.asarray(x).shape)
